# revision 1
# baseline (speedup 1.0000x reference)
"""Trainium2 Bass kernel for pairwise-scores CoreNet.

scores[i, j] = (e_i @ wa) + (e_j @ wb) + sum_d wc_d * |e_id - e_jd| + b

Strategy (8 cores, row-sharded i):
  * Each core holds the full embedding table transposed into SBUF as
    E_T[d partitions, j free], pre-scaled by |wc_d| and cast to bf16.
  * For each of the core's 128 rows i, one fused DVE tensor_scalar per
    128-d-tile computes |E~_dj - E~_di|  (op0=subtract with per-partition
    scalar e_i, op1=abs_max with 0)  ->  bf16 [128, N] tile at 4x mode.
  * The weighted d-reduction runs on the PE as an accumulating matvec:
    stationary = sign(wc) placed in a sliding one-hot column window so row
    i of PSUM accumulates  sum_d sign(wc_d)*|...| .  PSUM [128 i, 512 j]
    collects the whole output block in 2 banks.
  * sb row bias is folded in via a K=1 ones-column matmul (PSUM preload),
    sa + b are folded into the PSUM->SBUF copy (per-partition scalar add).
"""

import sys

sys.path.insert(0, "/opt/trn_rl_repo")

from contextlib import ExitStack

import numpy as np

import concourse.bass as bass
import concourse.mybir as mybir
import concourse.tile as tile
from concourse import bacc
from concourse.bass_utils import run_bass_kernel_spmd

F32 = mybir.dt.float32
BF16 = mybir.dt.bfloat16
Alu = mybir.AluOpType

N_CORES = 8


def build_program(n: int, d: int, r: int) -> bass.Bass:
    """Build the per-core Bass program.

    n: total rows/cols of the score matrix (full embedding count)
    d: embedding dim
    r: rows computed by this core (<= 128)
    """
    assert n % 512 == 0 and d % 128 == 0 and r <= 128
    H = d // 128          # number of 128-row d-tiles
    JB = n // 128         # transpose blocks along j
    JC = n // 512         # psum chunks along j

    nc = bacc.Bacc("TRN2", target_bir_lowering=False, debug=False)

    e_dram = nc.dram_tensor("emb", [n, d], F32, kind="ExternalInput")
    eb_dram = nc.dram_tensor("embblk", [r, d], F32, kind="ExternalInput")
    w_dram = nc.dram_tensor("wmat", [3 * d, 1], F32, kind="ExternalInput")
    b_dram = nc.dram_tensor("bvec", [1], F32, kind="ExternalInput")
    out_dram = nc.dram_tensor("scores", [r, n], F32, kind="ExternalOutput")
    sb_dram = nc.dram_tensor("sb_scratch", [JB, 128], F32)

    ident_dram = nc.inline_tensor(np.eye(128, dtype=np.float32), name="ident128")

    with tile.TileContext(nc) as tc, ExitStack() as ctx:
        const = ctx.enter_context(tc.tile_pool(name="const", bufs=1))
        work = ctx.enter_context(tc.tile_pool(name="work", bufs=2))
        absd_pool = ctx.enter_context(tc.tile_pool(name="absd", bufs=2))
        ps_acc = ctx.enter_context(tc.tile_pool(name="psacc", bufs=1, space="PSUM"))
        ps_t = ctx.enter_context(tc.tile_pool(name="pst", bufs=2, space="PSUM"))

        # ---------------- loads ----------------
        # PE (LDWEIGHTS) instructions can encode only one sync-wait, so every
        # tensor a PE op reads is staged through a DVE copy: PE then waits on
        # the single DVE semaphore instead of multiple DMA-queue semaphores.
        ident_raw = const.tile([128, 128], F32)
        nc.sync.dma_start(out=ident_raw[:, :], in_=ident_dram.ap())
        ident = const.tile([128, 128], F32)
        nc.vector.tensor_copy(ident[:, :], ident_raw[:, :])

        # e_nat[p, blk*d + c] = E[blk*128 + p, c]
        e_raw = const.tile([128, JB * d], F32)
        nc.sync.dma_start(
            out=e_raw[:, :].rearrange("p (b c) -> p b c", c=d),
            in_=e_dram.ap().rearrange("(b p) c -> p b c", p=128),
        )
        e_nat = const.tile([128, JB * d], F32)
        nc.vector.tensor_copy(e_nat[:, :], e_raw[:, :])

        eb_raw = const.tile([128, d], F32)
        if r < 128:
            nc.vector.memset(eb_raw[:, :], 0.0)
        nc.sync.dma_start(out=eb_raw[:r, :], in_=eb_dram.ap())
        eb_nat = const.tile([128, d], F32)
        nc.vector.tensor_copy(eb_nat[:, :], eb_raw[:, :])

        # w_all[p, k] = W[k*128 + p, 0],  k in [0, 3H)
        w_all = const.tile([128, 3 * H], F32)
        nc.sync.dma_start(
            out=w_all[:, :],
            in_=w_dram.ap().rearrange("(k p) one -> p (k one)", p=128),
        )

        # broadcast wa|wb across partitions: wab[p, c] = W[c, 0], c in [0, 2d)
        wab = const.tile([128, 2 * d], F32)
        nc.sync.dma_start(
            out=wab[:, :],
            in_=w_dram.ap()[0 : 2 * d, 0:1].transpose([1, 0]).broadcast_to([128, 2 * d]),
        )

        b_raw = const.tile([1, 1], F32)
        nc.sync.dma_start(out=b_raw[0:1, 0:1], in_=b_dram.ap()[None, :])
        b_sb = const.tile([1, 1], F32)
        nc.vector.tensor_copy(b_sb[0:1, 0:1], b_raw[0:1, 0:1])

        # ---------------- derived weights ----------------
        wabs = const.tile([128, H], F32)   # |wc| per d-partition, col h = d-tile h
        nc.scalar.activation(
            wabs[:, :], w_all[:, 2 * H : 3 * H], mybir.ActivationFunctionType.Abs,
        )
        sgn = const.tile([128, H], BF16)   # sign(wc) in {-1, 0, +1}
        nc.scalar.sign(sgn[:, :], w_all[:, 2 * H : 3 * H])
        sgn32 = const.tile([128, H], F32)
        nc.scalar.sign(sgn32[:, :], w_all[:, 2 * H : 3 * H])

        # sliding stationary windows: win[:, h*256 + 128] = sgn[:, h], else 0
        win = const.tile([128, H * 256], BF16)
        nc.vector.memset(win[:, :], 0.0)
        for h in range(H):
            nc.vector.tensor_copy(win[:, h * 256 + 128 : h * 256 + 129], sgn[:, h : h + 1])

        # ---------------- transpose + scale the table ----------------
        # e_t[h][dp, j] = |wc|_{h*128+dp} * E[j, h*128+dp]   (bf16)
        e_t = [const.tile([128, n], BF16, name=f"e_t{h}", tag=f"e_t{h}") for h in range(H)]
        for blk in range(JB):
            for h in range(H):
                pt = ps_t.tile([128, 128], F32, tag="pt")
                nc.tensor.transpose(
                    pt[:, :], e_nat[:, blk * d + h * 128 : blk * d + (h + 1) * 128],
                    ident[:, :],
                )
                nc.vector.tensor_scalar(
                    out=e_t[h][:, blk * 128 : (blk + 1) * 128], in0=pt[:, :],
                    scalar1=wabs[:, h : h + 1], scalar2=None, op0=Alu.mult,
                )

        # per-i scalar table (fp32): ebt[:, h*128 + i] = |wc| * E_blk[i, h*128+dp]
        ebt = const.tile([128, H * 128], F32)
        for h in range(H):
            pt = ps_t.tile([128, 128], F32, tag="pt")
            nc.tensor.transpose(
                pt[:, :], eb_nat[:, h * 128 : (h + 1) * 128], ident[:, :]
            )
            nc.vector.tensor_scalar(
                out=ebt[:, h * 128 : (h + 1) * 128], in0=pt[:, :],
                scalar1=wabs[:, h : h + 1], scalar2=None, op0=Alu.mult,
            )

        neg_ones_row = const.tile([1, 512], BF16)
        nc.vector.memset(neg_ones_row[0:1, :], -1.0)

        # q_j = sum_d sgn_d * E~[d, j]  (PE matvec), and the same for this
        # core's i block from the fp32 scalar table.
        ps_q = ctx.enter_context(tc.tile_pool(name="psq", bufs=1, space="PSUM"))
        q_psum = ps_q.tile([1, n], F32)
        for jc in range(JC):
            for h in range(H):
                nc.tensor.matmul(
                    q_psum[0:1, jc * 512 : (jc + 1) * 512],
                    lhsT=sgn[:, h : h + 1],
                    rhs=e_t[h][:, jc * 512 : (jc + 1) * 512],
                    start=(h == 0), stop=(h == H - 1), skip_group_check=True,
                )
        qb_psum = ps_q.tile([1, 128], F32)
        for h in range(H):
            nc.tensor.matmul(
                qb_psum[0:1, :],
                lhsT=sgn32[:, h : h + 1],
                rhs=ebt[:, h * 128 : (h + 1) * 128],
                start=(h == 0), stop=(h == H - 1), skip_group_check=True,
            )
        qb_row = const.tile([1, 128], BF16)
        nc.vector.tensor_copy(qb_row[0:1, :], qb_psum[0:1, :])
        q_row_f = const.tile([1, n], F32)
        nc.vector.tensor_copy(q_row_f[0:1, :], q_psum[0:1, :])

        # ---------------- sa / sb (fp32, DVE mul then free-dim reduce) ----------------
        sb_nat = const.tile([128, JB], F32)
        for blk in range(JB):
            scr = work.tile([128, d], F32, tag="scr")
            nc.vector.tensor_tensor(
                out=scr[:, :],
                in0=e_nat[:, blk * d : (blk + 1) * d],
                in1=wab[:, d : 2 * d],
                op=Alu.mult,
            )
            nc.vector.tensor_reduce(
                out=sb_nat[:, blk : blk + 1], in_=scr[:, :],
                axis=mybir.AxisListType.X, op=Alu.add,
            )
        sa_col = const.tile([128, 1], F32)
        scr = work.tile([128, d], F32, tag="scr")
        nc.vector.tensor_tensor(
            out=scr[:, :], in0=eb_nat[:, :], in1=wab[:, 0:d], op=Alu.mult,
        )
        nc.vector.tensor_reduce(
            out=sa_col[:, :], in_=scr[:, :], axis=mybir.AxisListType.X, op=Alu.add,
        )

        # sb: [128, JB] column layout -> DRAM bounce -> [1, n] row (bf16, +b)
        nc.sync.dma_start(out=sb_dram.ap().transpose([1, 0]), in_=sb_nat[:, :])
        sb_row_f = const.tile([1, n], F32)
        nc.sync.dma_start(
            out=sb_row_f[0:1, :], in_=sb_dram.ap().rearrange("b p -> (b p)")[None, :]
        )
        sbq_f = const.tile([1, n], F32)
        nc.vector.tensor_tensor(
            out=sbq_f[0:1, :], in0=sb_row_f[0:1, :], in1=q_row_f[0:1, :],
            op=Alu.subtract,
        )
        sb_row = const.tile([1, n], BF16)
        nc.vector.tensor_scalar(
            out=sb_row[0:1, :], in0=sbq_f[0:1, :],
            scalar1=b_sb[0:1, 0:1], scalar2=None, op0=Alu.add,
        )

        ones_col = const.tile([1, 128], BF16)
        nc.vector.memset(ones_col[0:1, :], 1.0)

        # ---------------- PSUM preload with sb ----------------
        psums = [
            ps_acc.tile([128, 512], F32, name=f"acc{jc}", tag=f"acc{jc}")
            for jc in range(JC)
        ]
        for jc in range(JC):
            nc.tensor.matmul(
                psums[jc][:, :],
                lhsT=ones_col[0:1, :],
                rhs=sb_row[0:1, jc * 512 : (jc + 1) * 512],
                start=True, stop=False, skip_group_check=True,
            )
            nc.tensor.matmul(
                psums[jc][:, :],
                lhsT=qb_row[0:1, :],
                rhs=neg_ones_row[0:1, :],
                start=False, stop=False, skip_group_check=True,
            )

        # ---------------- main loop ----------------
        for i in range(r):
            absd = [absd_pool.tile([128, n], BF16, name=f"absd{h}", tag=f"absd{h}") for h in range(H)]
            for h in range(H):
                # 2*max(E~_dj, E~_di); |a-b| = 2*max(a,b) - a - b, the linear
                # corrections are folded into the PSUM preload (q terms).
                nc.vector.tensor_scalar(
                    out=absd[h][:, :], in0=e_t[h][:, :],
                    scalar1=ebt[:, h * 128 + i : h * 128 + i + 1],
                    scalar2=2.0, op0=Alu.max, op1=Alu.mult,
                )
            last = i == r - 1
            for h in range(H):
                lw = win[:, h * 256 + 128 - i : h * 256 + 256 - i]
                for jc in range(JC):
                    nc.tensor.matmul(
                        psums[jc][:, :],
                        lhsT=lw,
                        rhs=absd[h][:, jc * 512 : (jc + 1) * 512],
                        start=False, stop=(last and h == H - 1),
                        skip_group_check=True,
                    )

        # ---------------- epilogue: += sa + (b already in sb), store ----------------
        out_s = const.tile([128, n], F32)
        for jc in range(JC):
            nc.vector.tensor_scalar(
                out=out_s[:, jc * 512 : (jc + 1) * 512], in0=psums[jc][:, :],
                scalar1=sa_col[:, :], scalar2=None, op0=Alu.add,
            )
        nc.sync.dma_start(out=out_dram.ap(), in_=out_s[:r, :])

    nc.finalize()
    return nc


_CACHE: dict = {}


def _get_program(n: int, d: int, r: int) -> bass.Bass:
    key = (n, d, r)
    if key not in _CACHE:
        _CACHE[key] = build_program(n, d, r)
    return _CACHE[key]


def kernel(**inputs: np.ndarray) -> np.ndarray:
    emb = np.ascontiguousarray(np.asarray(inputs["utterance_embeddings"], dtype=np.float32))
    W = np.ascontiguousarray(np.asarray(inputs["W"], dtype=np.float32))
    b = np.ascontiguousarray(np.asarray(inputs["b"], dtype=np.float32))
    n, d = emb.shape
    assert n % N_CORES == 0
    r = n // N_CORES

    nc = _get_program(n, d, r)
    in_maps = [
        {
            "emb": emb,
            "embblk": np.ascontiguousarray(emb[c * r : (c + 1) * r]),
            "wmat": W,
            "bvec": b,
        }
        for c in range(N_CORES)
    ]
    res = run_bass_kernel_spmd(nc, in_maps, list(range(N_CORES)))
    blocks = [res.results[c]["scores"] for c in range(N_CORES)]
    return np.concatenate(blocks, axis=0).astype(np.float32)


if __name__ == "__main__":
    rng = np.random.default_rng(0)
    n, d = 1024, 256
    emb = rng.standard_normal((n, d), dtype=np.float32)
    W = (rng.standard_normal((3 * d, 1), dtype=np.float32) / np.sqrt(3 * d)).astype(np.float32)
    b = np.zeros((1,), dtype=np.float32)
    out = kernel(utterance_embeddings=emb, W=W, b=b)
    print(out.shape, out.dtype)



# revision 2
# speedup vs baseline: 1.3584x; 1.3584x over previous
"""Trainium2 Bass kernel for pairwise-scores CoreNet.

scores[i, j] = (e_i @ wa) + (e_j @ wb) + sum_d wc_d * |e_id - e_jd| + b

The |.| term is symmetric in (i, j), so only the upper triangle is computed
on-device and the lower triangle is mirrored on the host during unshard:
  scores[i, j<i] = scores[j, i] + (sa_i - sb_i) - (sa_j - sb_j).

Sharding (8 cores): rows are snaked in 16-row blocks so every core gets the
same multiset of row lengths: core c owns rows {16m + c} u {16m + 15 - c}.
Each device row k (global row i, m = k//2) computes columns j in
[16*m, 1024) - a superset of the triangle row [i, 1024) - so the program is
identical on all cores; only the gathered `embT_own` input differs.

Per-core dataflow, with e~ = |wc| * e laid out [d partitions, j free]:
  * Production of m-tiles t[d, j]: DVE rows use max(e~_dj, e~_di)
    (tensor_scalar, 4x mode); ACT rows use relu(e~_dj - e~_di)
    (activation with per-partition bias). Rows are split between the two
    engines by a greedy makespan balance.
  * Reduction over d rides the PE: stationary = sliding one-hot window whose
    column k holds 2*sign(wc) for the 128-d tile h, so PSUM row k
    accumulates 2 * sum_d sgn_d * t[d, j].  Identities:
      sgn*|a-b| = 2*sgn*max(a,b) - sgn*a - sgn*b      (DVE rows)
      sgn*|a-b| = 2*sgn*relu(b-a) + sgn*a - sgn*b     (ACT rows)
    The -q_j column term (q = sum_d sgn*e~) plus sb_j + b is preloaded into
    PSUM; the per-row +/-q_i + sa_i lands in the epilogue via a [128,1]
    column computed on-PE with a sign-folded stationary.
"""

import sys

sys.path.insert(0, "/opt/trn_rl_repo")

from contextlib import ExitStack

import numpy as np

import concourse.bass as bass
import concourse.mybir as mybir
import concourse.tile as tile
from concourse import bacc
from concourse.bass_utils import run_bass_kernel_spmd

F32 = mybir.dt.float32
BF16 = mybir.dt.bfloat16
Alu = mybir.AluOpType
Act = mybir.ActivationFunctionType

N_CORES = 8
N = 1024
D = 256
R = 128  # rows per core

# Greedy makespan split of rows between DVE and ACT producers.
# Per-(row, h-tile) op costs in ns (measured): DVE 130 + 0.2604*L at 4x,
# ACT 279 + 0.711*L.
def _assign_rows() -> list[str]:
    assign = []
    dve_t = 0.0
    act_t = 0.0
    for k in range(R):
        L = N - 16 * (k // 2)
        cd = 2 * (130.0 + 0.2604 * L)
        ca = 2 * (279.0 + 0.711 * L)
        if act_t + ca <= dve_t + cd:
            assign.append("act")
            act_t += ca
        else:
            assign.append("dve")
            dve_t += cd
    return assign


ASSIGN = _assign_rows()


def build_program() -> bass.Bass:
    nc = bacc.Bacc("TRN2", target_bir_lowering=False, debug=False)

    et_dram = nc.dram_tensor("embT", [D, N], F32, kind="ExternalInput")
    own_dram = nc.dram_tensor("embTown", [D, R], F32, kind="ExternalInput")
    w_dram = nc.dram_tensor("wmat", [3 * D, 1], F32, kind="ExternalInput")
    b_dram = nc.dram_tensor("bvec", [1], F32, kind="ExternalInput")
    out_dram = nc.dram_tensor("scores", [R, N], F32, kind="ExternalOutput")

    # smat[h][d, k] = -1 if row k produced in max-form (DVE) else +1
    svec = np.array([1.0 if a == "act" else -1.0 for a in ASSIGN], dtype=np.float32)
    smat_np = np.broadcast_to(svec[None, :], (128, R)).copy()
    smat_dram = nc.inline_tensor(smat_np, name="smat")

    with tile.TileContext(nc) as tc, ExitStack() as ctx:
        const = ctx.enter_context(tc.tile_pool(name="const", bufs=1))
        prod = ctx.enter_context(tc.tile_pool(name="prod", bufs=5))
        ps_acc = ctx.enter_context(tc.tile_pool(name="psacc", bufs=1, space="PSUM"))
        ps_aux = ctx.enter_context(tc.tile_pool(name="psaux", bufs=1, space="PSUM"))

        # ---------------- loads ----------------
        er_raw = [const.tile([128, N], F32, name=f"er{h}", tag=f"er{h}") for h in range(2)]
        for h in range(2):
            nc.sync.dma_start(out=er_raw[h][:, :], in_=et_dram.ap()[128 * h : 128 * (h + 1), :])
        own_raw = [const.tile([128, R], F32, name=f"ow{h}", tag=f"ow{h}") for h in range(2)]
        for h in range(2):
            nc.sync.dma_start(out=own_raw[h][:, :], in_=own_dram.ap()[128 * h : 128 * (h + 1), :])
        # w_all[p, k] = W[k*128 + p, 0], k in [0, 6)
        w_all = const.tile([128, 6], F32)
        nc.sync.dma_start(
            out=w_all[:, :], in_=w_dram.ap().rearrange("(k p) one -> p (k one)", p=128)
        )
        b_raw = const.tile([1, 1], F32)
        nc.sync.dma_start(out=b_raw[0:1, 0:1], in_=b_dram.ap()[None, :])
        b_sb = const.tile([1, 1], F32)
        nc.vector.tensor_copy(b_sb[0:1, 0:1], b_raw[0:1, 0:1])
        smat_raw = const.tile([128, R], F32)
        nc.sync.dma_start(out=smat_raw[:, :], in_=smat_dram.ap())

        # ---------------- derived weights (tiny) ----------------
        # columns of w_all: 0,1 = wa | 2,3 = wb | 4,5 = wc  (128-dim tiles)
        wabs = const.tile([128, 2], F32)
        nc.scalar.activation(wabs[:, :], w_all[:, 4:6], Act.Abs)
        sgnf = const.tile([128, 2], F32)
        nc.scalar.sign(sgnf[:, :], w_all[:, 4:6])
        sgn1 = const.tile([128, 2], BF16)
        nc.vector.tensor_copy(sgn1[:, :], sgnf[:, :])
        msgn = const.tile([128, 2], BF16)
        nc.vector.tensor_scalar(
            out=msgn[:, :], in0=sgnf[:, :], scalar1=-1.0, scalar2=None, op0=Alu.mult
        )
        sgn2 = const.tile([128, 2], BF16)
        nc.vector.tensor_scalar(
            out=sgn2[:, :], in0=sgnf[:, :], scalar1=2.0, scalar2=None, op0=Alu.mult
        )
        wacol = const.tile([128, 2], BF16)
        nc.vector.tensor_copy(wacol[:, :], w_all[:, 0:2])
        wbcol = const.tile([128, 2], BF16)
        nc.vector.tensor_copy(wbcol[:, :], w_all[:, 2:4])

        # ---------------- main tables ----------------
        e_t = [const.tile([128, N], BF16, name=f"et{h}", tag=f"et{h}") for h in range(2)]
        ebr = [const.tile([128, N], BF16, name=f"ebr{h}", tag=f"ebr{h}") for h in range(2)]
        own_sc = [const.tile([128, R], F32, name=f"os{h}", tag=f"os{h}") for h in range(2)]
        nown = [const.tile([128, R], F32, name=f"no{h}", tag=f"no{h}") for h in range(2)]
        own_bf = [const.tile([128, R], BF16, name=f"ob{h}", tag=f"ob{h}") for h in range(2)]
        ownq = [const.tile([128, R], BF16, name=f"oq{h}", tag=f"oq{h}") for h in range(2)]
        for h in range(2):
            nc.vector.tensor_scalar(
                out=e_t[h][:, :], in0=er_raw[h][:, :],
                scalar1=wabs[:, h : h + 1], scalar2=None, op0=Alu.mult,
            )
            nc.vector.tensor_copy(ebr[h][:, :], er_raw[h][:, :])
            nc.vector.tensor_scalar(
                out=own_sc[h][:, :], in0=own_raw[h][:, :],
                scalar1=wabs[:, h : h + 1], scalar2=None, op0=Alu.mult,
            )
            nc.vector.tensor_scalar(
                out=nown[h][:, :], in0=own_sc[h][:, :],
                scalar1=-1.0, scalar2=None, op0=Alu.mult,
            )
            nc.vector.tensor_copy(own_bf[h][:, :], own_raw[h][:, :])
            nc.vector.tensor_tensor(
                out=ownq[h][:, :], in0=own_sc[h][:, :], in1=smat_raw[:, :], op=Alu.mult
            )

        # sliding one-hot windows: win2[h][:, 128] = 2*sgn_h
        win2 = [const.tile([128, 256], BF16, name=f"w2{h}", tag=f"w2{h}") for h in range(2)]
        for h in range(2):
            nc.vector.memset(win2[h][:, :], 0.0)
            nc.vector.tensor_copy(win2[h][:, 128:129], sgn2[:, h : h + 1])
        ones_row = const.tile([1, 128], BF16)
        nc.vector.memset(ones_row[0:1, :], 1.0)

        # ---------------- column preload vector v_j = sb_j + b - q_j ------
        psv = [ps_aux.tile([1, 512], F32, name=f"psv{jc}", tag=f"psv{jc}") for jc in range(2)]
        for jc in range(2):
            sl = slice(512 * jc, 512 * (jc + 1))
            for h in range(2):
                nc.tensor.matmul(
                    psv[jc][0:1, :], lhsT=wbcol[:, h : h + 1], rhs=ebr[h][:, sl],
                    start=(h == 0), stop=False, skip_group_check=True,
                )
            for h in range(2):
                nc.tensor.matmul(
                    psv[jc][0:1, :], lhsT=msgn[:, h : h + 1], rhs=e_t[h][:, sl],
                    start=False, stop=(h == 1), skip_group_check=True,
                )
        v_row = const.tile([1, N], BF16)
        for jc in range(2):
            nc.vector.tensor_scalar(
                out=v_row[0:1, 512 * jc : 512 * (jc + 1)], in0=psv[jc][0:1, :],
                scalar1=b_sb[0:1, 0:1], scalar2=None, op0=Alu.add,
            )

        # ---------------- row vector u_k = sa_i + s_k * q_i ----------------
        psu = ps_aux.tile([128, 1], F32)
        for h in range(2):
            nc.tensor.matmul(
                psu[:, :], lhsT=own_bf[h][:, :], rhs=wacol[:, h : h + 1],
                start=(h == 0), stop=False, skip_group_check=True,
            )
        for h in range(2):
            nc.tensor.matmul(
                psu[:, :], lhsT=ownq[h][:, :], rhs=sgn1[:, h : h + 1],
                start=False, stop=(h == 1), skip_group_check=True,
            )
        u_col = const.tile([128, 1], F32)
        nc.vector.tensor_copy(u_col[:, :], psu[:, :])

        # ---------------- PSUM preload ----------------
        psums = [
            ps_acc.tile([128, 512], F32, name=f"acc{jc}", tag=f"acc{jc}")
            for jc in range(2)
        ]
        for jc in range(2):
            nc.tensor.matmul(
                psums[jc][:, :], lhsT=ones_row[0:1, :],
                rhs=v_row[0:1, 512 * jc : 512 * (jc + 1)],
                start=True, stop=False, skip_group_check=True,
            )

        # ---------------- main loop ----------------
        out_s = const.tile([128, N], F32)
        for k in range(R):
            m = k // 2
            j0 = 16 * m
            eng = ASSIGN[k]
            for h in range(2):
                a = prod.tile([128, N], BF16, name=f"ab_{eng}{h}", tag=f"ab_{eng}{h}")
                if eng == "dve":
                    nc.vector.tensor_scalar(
                        out=a[:, j0:], in0=e_t[h][:, j0:],
                        scalar1=own_sc[h][:, k : k + 1], scalar2=None, op0=Alu.max,
                    )
                else:
                    nc.scalar.activation(
                        a[:, j0:], e_t[h][:, j0:], Act.Relu,
                        bias=nown[h][:, k : k + 1], scale=1.0,
                    )
                lw = win2[h][:, 128 - k : 256 - k]
                if k < 64:
                    nc.tensor.matmul(
                        psums[0][:, j0:512], lhsT=lw, rhs=a[:, j0:512],
                        start=False, stop=(k == 63 and h == 1),
                        skip_group_check=True,
                    )
                    nc.tensor.matmul(
                        psums[1][:, :], lhsT=lw, rhs=a[:, 512:1024],
                        start=False, stop=False, skip_group_check=True,
                    )
                else:
                    nc.tensor.matmul(
                        psums[1][:, j0 - 512 : 512], lhsT=lw, rhs=a[:, j0:1024],
                        start=False, stop=(k == 127 and h == 1),
                        skip_group_check=True,
                    )
            if k == 63:
                # psums[0] complete: drain its half early
                nc.vector.tensor_scalar(
                    out=out_s[:, 0:512], in0=psums[0][:, :],
                    scalar1=u_col[:, :], scalar2=None, op0=Alu.add,
                )
                nc.sync.dma_start(out=out_dram.ap()[:, 0:512], in_=out_s[:, 0:512])

        nc.vector.tensor_scalar(
            out=out_s[:, 512:1024], in0=psums[1][:, :],
            scalar1=u_col[:, :], scalar2=None, op0=Alu.add,
        )
        nc.sync.dma_start(out=out_dram.ap()[:, 512:1024], in_=out_s[:, 512:1024])

    nc.finalize()
    return nc


_CACHE: dict = {}


def _get_program() -> bass.Bass:
    if "p" not in _CACHE:
        _CACHE["p"] = build_program()
    return _CACHE["p"]


def core_rows(c: int) -> list[int]:
    return sorted([16 * m + c for m in range(64)] + [16 * m + 15 - c for m in range(64)])


def make_in_maps(emb: np.ndarray, W: np.ndarray, b: np.ndarray) -> list[dict]:
    embT = np.ascontiguousarray(emb.T.astype(np.float32))
    maps = []
    for c in range(N_CORES):
        rows = core_rows(c)
        maps.append(
            {
                "embT": embT,
                "embTown": np.ascontiguousarray(embT[:, rows]),
                "wmat": W.astype(np.float32),
                "bvec": b.astype(np.float32),
            }
        )
    return maps


def kernel(**inputs: np.ndarray) -> np.ndarray:
    emb = np.ascontiguousarray(np.asarray(inputs["utterance_embeddings"], dtype=np.float32))
    W = np.ascontiguousarray(np.asarray(inputs["W"], dtype=np.float32))
    b = np.ascontiguousarray(np.asarray(inputs["b"], dtype=np.float32))
    n, d = emb.shape
    assert (n, d) == (N, D)

    nc = _get_program()
    res = run_bass_kernel_spmd(nc, make_in_maps(emb, W, b), list(range(N_CORES)))

    S = np.empty((N, N), dtype=np.float32)
    for c in range(N_CORES):
        S[core_rows(c), :] = res.results[c]["scores"]

    # mirror the not-computed region: row i holds valid cols j >= 16*(i//16)
    w = W[:, 0]
    delta = emb @ (w[:d] - w[d : 2 * d])  # sa - sb
    jj = np.arange(N)
    mask = (jj[None, :] // 16) >= (jj[:, None] // 16)
    S = np.where(mask, S, S.T + delta[:, None] - delta[None, :])
    return S.astype(np.float32)


if __name__ == "__main__":
    rng = np.random.default_rng(0)
    emb = rng.standard_normal((N, D), dtype=np.float32)
    W = (rng.standard_normal((3 * D, 1), dtype=np.float32) / np.sqrt(3 * D)).astype(np.float32)
    b = np.zeros((1,), dtype=np.float32)
    out = kernel(utterance_embeddings=emb, W=W, b=b)
    print(out.shape, out.dtype)


# revision 9
# speedup vs baseline: 1.7709x; 1.3036x over previous
"""Trainium2 Bass kernel for pairwise-scores CoreNet.

scores[i, j] = (e_i @ wa) + (e_j @ wb) + sum_d wc_d * |e_id - e_jd| + b

The |.| term is symmetric in (i, j): only the upper triangle is computed
on-device; the host mirrors the rest during unshard:
  scores[i, j<i] = scores[j, i] + (sa_i - sb_i) - (sa_j - sb_j).

Sharding (8 cores): rows are snaked in 16-row blocks so every core gets the
same multiset of row lengths: core c owns rows {16m + c} u {16m + 15 - c}.
Device row k (global row i, m = k//2) computes columns j in [16*m, 1024) -
a superset of [i, 1024) - so one program serves all cores; only the gathered
`embTown` input differs per core.

Per-core dataflow, e~ = |wc| * e laid out [d partitions, j free] (bf16):
  * Production of m-tiles t[d, j]: DVE rows compute max(e~_dj, e~_di)
    (tensor_scalar, 4x mode); ACT rows compute relu(e~_dj - e~_di)
    (activation, per-partition bias).  Rows are split between the engines by
    greedy makespan balancing (costs fitted from HW traces).
  * The d-reduction rides the PE: stationary = sliding one-hot window whose
    column k holds 2*sign(wc) for d-tile h, accumulating into PSUM row k:
      sgn*|a-b| = 2*sgn*max(a,b) - sgn*a - sgn*b      (DVE rows)
      sgn*|a-b| = 2*sgn*relu(b-a) + sgn*a - sgn*b     (ACT rows)
  * Since sgn_d*|wc_d| = wc_d, the linear corrections collapse:
      column term  v_j = b + sum_d (wb_d - wc_d) e_dj   (4 PE matvecs on raw e)
      row term     u_k = sum_d (wa_d + s_k wc_d) e_di   (2 PE matvecs,
                   s_k = -1 for DVE rows, +1 for ACT rows, folded into the
                   host-prepped `uw` weight matrix)
    v is broadcast into a spare PSUM pair off the critical path; the epilogue
    fuses psum + u + v in one scalar_tensor_tensor per 512-column half.  The
    first half drains at k=63, overlapping the remaining matmul stream.
"""

import sys

sys.path.insert(0, "/opt/trn_rl_repo")

from contextlib import ExitStack

import ml_dtypes
import numpy as np

import concourse.bass as bass
import concourse.mybir as mybir
import concourse.tile as tile
from concourse import bacc
from concourse.bass_utils import run_bass_kernel_spmd

F32 = mybir.dt.float32
BF16 = mybir.dt.bfloat16
Alu = mybir.AluOpType
Act = mybir.ActivationFunctionType

N_CORES = 8
N = 1024
D = 256
R = 128  # rows per core


def _assign_rows() -> list[str]:
    """Greedy makespan split of rows between the DVE and ACT producers.
    Per-(row, h-tile) engine-busy costs fitted from HW traces."""
    assign = []
    dve_t = 0.0
    act_t = 0.0
    for k in range(R):
        L = N - 16 * (k // 2)
        cd = 2 * (260.0 + 0.264 * L)
        ca = 2 * (367.0 + 0.829 * L)
        if act_t + ca <= dve_t + cd:
            assign.append("act")
            act_t += ca
        else:
            assign.append("dve")
            dve_t += cd
    return assign


ASSIGN = _assign_rows()


def build_program() -> bass.Bass:
    nc = bacc.Bacc("TRN2", target_bir_lowering=False, debug=False)

    et_dram = nc.dram_tensor("embT", [D, N], BF16, kind="ExternalInput")
    own_dram = nc.dram_tensor("embTown", [D, R], BF16, kind="ExternalInput")
    # waux_bf cols: 0,1 = wb - wc | 2,3 = 2*sign(wc)   (128-d tiles h=0,1)
    wauxb_dram = nc.dram_tensor("wauxb", [128, 4], BF16, kind="ExternalInput")
    # waux_f cols: 0,1 = |wc| | 2,3 = -|wc|
    wauxf_dram = nc.dram_tensor("wauxf", [128, 4], F32, kind="ExternalInput")
    # uw[d, k] = wa_d + s_k * wc_d  (s_k from ASSIGN)
    uw_dram = nc.dram_tensor("uw", [D, R], BF16, kind="ExternalInput")
    b_dram = nc.dram_tensor("bvec", [1], F32, kind="ExternalInput")
    out_dram = nc.dram_tensor("scores", [R, N], F32, kind="ExternalOutput")

    with tile.TileContext(nc) as tc, ExitStack() as ctx:
        const = ctx.enter_context(tc.tile_pool(name="const", bufs=1))
        prod = ctx.enter_context(tc.tile_pool(name="prod", bufs=10))
        ps_acc = ctx.enter_context(tc.tile_pool(name="psacc", bufs=1, space="PSUM"))
        ps_aux = ctx.enter_context(tc.tile_pool(name="psaux", bufs=1, space="PSUM"))

        # ---------------- loads (two parallel DMA queues) ----------------
        wauxb = const.tile([128, 4], BF16)
        nc.sync.dma_start(out=wauxb[:, :], in_=wauxb_dram.ap())
        wauxf = const.tile([128, 4], F32)
        nc.sync.dma_start(out=wauxf[:, :], in_=wauxf_dram.ap())
        b_raw = const.tile([1, 1], F32)
        nc.sync.dma_start(out=b_raw[0:1, 0:1], in_=b_dram.ap()[None, :])
        ebr = [const.tile([128, N], BF16, name=f"ebr{h}", tag=f"ebr{h}") for h in range(2)]
        nc.sync.dma_start(out=ebr[0][:, :], in_=et_dram.ap()[0:128, :])
        own_raw = [const.tile([128, R], BF16, name=f"ow{h}", tag=f"ow{h}") for h in range(2)]
        uwt = [const.tile([128, R], BF16, name=f"uw{h}", tag=f"uw{h}") for h in range(2)]
        for h in range(2):
            nc.scalar.dma_start(out=own_raw[h][:, :], in_=own_dram.ap()[128 * h : 128 * (h + 1), :])
            nc.scalar.dma_start(out=uwt[h][:, :], in_=uw_dram.ap()[128 * h : 128 * (h + 1), :])
        nc.scalar.dma_start(out=ebr[1][:, :], in_=et_dram.ap()[128:256, :])

        # ---------------- tables ----------------
        e_t = [const.tile([128, N], BF16, name=f"et{h}", tag=f"et{h}") for h in range(2)]
        own_sc = [const.tile([128, R], F32, name=f"os{h}", tag=f"os{h}") for h in range(2)]
        nown = [const.tile([128, R], F32, name=f"no{h}", tag=f"no{h}") for h in range(2)]
        win2 = [const.tile([128, 256], BF16, name=f"w2{h}", tag=f"w2{h}") for h in range(2)]
        for h in range(2):
            nc.vector.tensor_scalar(
                out=e_t[h][:, :], in0=ebr[h][:, :],
                scalar1=wauxf[:, h : h + 1], scalar2=None, op0=Alu.mult,
            )
            nc.vector.tensor_scalar(
                out=own_sc[h][:, :], in0=own_raw[h][:, :],
                scalar1=wauxf[:, h : h + 1], scalar2=None, op0=Alu.mult,
            )
            nc.vector.memset(win2[h][:, :], 0.0)
            nc.vector.tensor_copy(win2[h][:, 128:129], wauxb[:, 2 + h : 3 + h])
            # nown = -|wc| * own_raw, built on ACT to pull the act-table load early
            nc.scalar.activation(
                nown[h][:, :], own_raw[h][:, :], Act.Copy,
                scale=wauxf[:, 2 + h : 3 + h],
            )
        b_sb = const.tile([1, 1], F32)
        nc.vector.tensor_copy(b_sb[0:1, 0:1], b_raw[0:1, 0:1])
        ones_row = const.tile([1, 128], BF16)
        nc.vector.memset(ones_row[0:1, :], 1.0)
        ones_col = const.tile([128, 1], BF16)
        nc.vector.memset(ones_col[:, :], 1.0)

        # ---------------- PSUM tiles ----------------
        psums = [
            ps_acc.tile([128, 512], F32, name=f"acc{jc}", tag=f"acc{jc}")
            for jc in range(2)
        ]
        psv = [ps_aux.tile([1, 512], F32, name=f"psv{jc}", tag=f"psv{jc}") for jc in range(2)]
        psv2 = [ps_aux.tile([128, 512], F32, name=f"pv2{jc}", tag=f"pv2{jc}") for jc in range(2)]
        psu = ps_aux.tile([128, 1], F32)

        # v_j = b + sum_d (wb - wc)_d e_dj : 4 matvecs on raw e (no DVE dep)
        for jc in range(2):
            sl = slice(512 * jc, 512 * (jc + 1))
            for h in range(2):
                nc.tensor.matmul(
                    psv[jc][0:1, :], lhsT=wauxb[:, h : h + 1], rhs=ebr[h][:, sl],
                    start=(h == 0), stop=(h == 1), skip_group_check=True,
                )
        v_row = const.tile([1, N], BF16)
        for jc in range(2):
            nc.scalar.activation(
                v_row[0:1, 512 * jc : 512 * (jc + 1)], psv[jc][0:1, :],
                Act.Identity, bias=b_sb[0:1, 0:1],
            )

        v_bc = const.tile([128, N], F32)
        u_col = const.tile([128, 1], F32)

        # ---------------- main loop ----------------
        out_s = const.tile([128, N], F32)
        for k in range(R):
            m = k // 2
            j0 = 16 * m
            eng = ASSIGN[k]
            for h in range(2):
                a = prod.tile([128, N], BF16, name=f"ab_{eng}{h}", tag=f"ab_{eng}{h}")
                if eng == "dve":
                    nc.vector.tensor_scalar(
                        out=a[:, j0:], in0=e_t[h][:, j0:],
                        scalar1=own_sc[h][:, k : k + 1], scalar2=None, op0=Alu.max,
                    )
                else:
                    nc.scalar.activation(
                        a[:, j0:], e_t[h][:, j0:], Act.Relu,
                        bias=nown[h][:, k : k + 1], scale=1.0,
                    )
                lw = win2[h][:, 128 - k : 256 - k]
                if k < 64:
                    nc.tensor.matmul(
                        psums[0][:, j0:512], lhsT=lw, rhs=a[:, j0:512],
                        start=(k == 0 and h == 0), stop=(k == 63 and h == 1),
                        skip_group_check=True,
                    )
                    nc.tensor.matmul(
                        psums[1][:, :], lhsT=lw, rhs=a[:, 512:1024],
                        start=(k == 0 and h == 0), stop=False,
                        skip_group_check=True,
                    )
                else:
                    nc.tensor.matmul(
                        psums[1][:, j0 - 512 : 512], lhsT=lw, rhs=a[:, j0:1024],
                        start=False, stop=(k == 127 and h == 1),
                        skip_group_check=True,
                    )
            if k == 1:
                # off-critical-path PE work: broadcast v into psv2
                for jc in range(2):
                    nc.tensor.matmul(
                        psv2[jc][:, :], lhsT=ones_row[0:1, :],
                        rhs=v_row[0:1, 512 * jc : 512 * (jc + 1)],
                        start=True, stop=True, skip_group_check=True,
                    )
            if k == 2:
                # u_k = sum_d (wa + s_k wc)_d e_d,i_k via sign-folded stationary
                for h in range(2):
                    ouw = const.tile([128, R], BF16, name=f"ouw{h}", tag=f"ouw{h}")
                    nc.vector.tensor_tensor(
                        out=ouw[:, :], in0=own_raw[h][:, :], in1=uwt[h][:, :],
                        op=Alu.mult,
                    )
                    nc.tensor.matmul(
                        psu[:, :], lhsT=ouw[:, :], rhs=ones_col[:, 0:1],
                        start=(h == 0), stop=(h == 1), skip_group_check=True,
                    )
            if k == 4:
                nc.vector.tensor_copy(u_col[:, :], psu[:, :])
                for jc in range(2):
                    nc.vector.tensor_copy(
                        v_bc[:, 512 * jc : 512 * (jc + 1)], psv2[jc][:, :]
                    )
            if k == 63:
                # psums[0] complete: drain the first half early
                nc.vector.scalar_tensor_tensor(
                    out=out_s[:, 0:512], in0=psums[0][:, :], scalar=u_col[:, :],
                    in1=v_bc[:, 0:512], op0=Alu.add, op1=Alu.add,
                )
                nc.sync.dma_start(out=out_dram.ap()[:, 0:512], in_=out_s[:, 0:512])

        nc.vector.scalar_tensor_tensor(
            out=out_s[:, 512:1024], in0=psums[1][:, :], scalar=u_col[:, :],
            in1=v_bc[:, 512:1024], op0=Alu.add, op1=Alu.add,
        )
        nc.sync.dma_start(out=out_dram.ap()[:, 512:768], in_=out_s[:, 512:768])
        nc.scalar.dma_start(out=out_dram.ap()[:, 768:1024], in_=out_s[:, 768:1024])

    nc.finalize()
    return nc


_CACHE: dict = {}


def _get_program() -> bass.Bass:
    if "p" not in _CACHE:
        _CACHE["p"] = build_program()
    return _CACHE["p"]


def core_rows(c: int) -> list[int]:
    return sorted([16 * m + c for m in range(64)] + [16 * m + 15 - c for m in range(64)])


def make_in_maps(emb: np.ndarray, W: np.ndarray, b: np.ndarray) -> list[dict]:
    bf = ml_dtypes.bfloat16
    embT = np.ascontiguousarray(emb.T.astype(np.float32)).astype(bf)
    w = W[:, 0].astype(np.float32)
    wa, wb, wc = w[:D], w[D : 2 * D], w[2 * D :]
    svec = np.array([1.0 if a == "act" else -1.0 for a in ASSIGN], dtype=np.float32)
    uw = (wa[:, None] + svec[None, :] * wc[:, None]).astype(bf)  # [D, R]
    wauxb = np.stack(
        [
            (wb - wc)[0:128],
            (wb - wc)[128:256],
            2.0 * np.sign(wc)[0:128],
            2.0 * np.sign(wc)[128:256],
        ],
        axis=1,
    ).astype(bf)  # [128, 4]
    wauxf = np.stack(
        [
            np.abs(wc)[0:128],
            np.abs(wc)[128:256],
            -np.abs(wc)[0:128],
            -np.abs(wc)[128:256],
        ],
        axis=1,
    ).astype(np.float32)
    maps = []
    for c in range(N_CORES):
        rows = core_rows(c)
        maps.append(
            {
                "embT": embT,
                "embTown": np.ascontiguousarray(embT[:, rows]),
                "wauxb": wauxb,
                "wauxf": wauxf,
                "uw": uw,
                "bvec": b.astype(np.float32),
            }
        )
    return maps


def kernel(**inputs: np.ndarray) -> np.ndarray:
    emb = np.ascontiguousarray(np.asarray(inputs["utterance_embeddings"], dtype=np.float32))
    W = np.ascontiguousarray(np.asarray(inputs["W"], dtype=np.float32))
    b = np.ascontiguousarray(np.asarray(inputs["b"], dtype=np.float32))
    n, d = emb.shape
    assert (n, d) == (N, D)

    nc = _get_program()
    res = run_bass_kernel_spmd(nc, make_in_maps(emb, W, b), list(range(N_CORES)))

    S = np.empty((N, N), dtype=np.float32)
    for c in range(N_CORES):
        S[core_rows(c), :] = res.results[c]["scores"]

    # mirror the not-computed region: row i holds valid cols j >= 16*(i//16)
    w = W[:, 0]
    delta = emb @ (w[:d] - w[d : 2 * d])  # sa - sb
    jj = np.arange(N)
    mask = (jj[None, :] // 16) >= (jj[:, None] // 16)
    S = np.where(mask, S, S.T + delta[:, None] - delta[None, :])
    return S.astype(np.float32)


if __name__ == "__main__":
    rng = np.random.default_rng(0)
    emb = rng.standard_normal((N, D), dtype=np.float32)
    W = (rng.standard_normal((3 * D, 1), dtype=np.float32) / np.sqrt(3 * D)).astype(np.float32)
    b = np.zeros((1,), dtype=np.float32)
    out = kernel(utterance_embeddings=emb, W=W, b=b)
    print(out.shape, out.dtype)
